# revision 15
# baseline (speedup 1.0000x reference)
"""Trainium2 Bass kernel for a 3x3 stride-1 pad-1 conv, NCHW (16,16,512,512) fp32.

Matches the reference semantics exactly:
  - effective weights: K flattened as (ki,kj,ci) but consumed as (ci,ki,kj):
      Weff[ki,kj,ci,co] = K.reshape(144,16)[ci*9 + ki*3 + kj, co]
  - last output row and column are zero.

Strategy: pure data parallel over the batch (2 images per core on 8 cores),
weights replicated. x is staged to the device as fp16 (host cast) and the
output is produced as fp16 on device (host upcast), halving both directions
of HBM traffic; accumulation stays fp32 in PSUM. Both device-side tensors
use a host-transposed [n, H, C, W] layout: the DRAM address of SBUF
partition p = hi*16+ci (resp. ho*16+co) is then p*W, a single
uniform-stride AP dim, which lets one 3-dim DMA move a whole block of
groups and makes every group window a contiguous 128KB DRAM extent.

Per core the conv runs as banded fp16 matmuls on the tensor engine:
  - output rows in groups of R=6; contraction K = 8 input rows x 16 c_in = 128
    partitions (rows s-1 .. s+6), M = 6 out rows x 16 c_out = 96;
  - partition layout hi*16+ci; data cols 16..527 of a 544-wide padded slot,
    pad cols 15/528 memset to zero;
  - the 3 kj taps are column-shifted slices of the padded row tile;
  - group starts: 0, 6, ..., 504, then a final overlapped group at 505
    (recomputed rows store near-identical bytes, so the overlap is benign);
  - groups are processed in blocks of 8: ONE input DMA loads all 8 group
    windows (the HWDGE issue path is a serialized ~630ns/instruction shared
    resource, so few fat DMAs beat many thin ones), 24 matmuls run
    back-to-back into 8 PSUM banks (keeping the PE continuously busy so it
    stays at its full-speed p-state), PSUM is evicted with an fp32->fp16
    cast split across the vector and scalar engines into an fp16 staging
    tile, and ONE store DMA scatters the whole block to DRAM.
"""

import numpy as np

import concourse.bass as bass
import concourse.mybir as mybir
import concourse.tile as tile
from concourse import bacc
from concourse.bass_utils import run_bass_kernel_spmd

F32 = mybir.dt.float32
F16 = mybir.dt.float16

C = 16  # channels (in == out)
W = 512  # image width
R = 6  # output rows per matmul group
RIN = R + 2  # input rows per group
M = R * C  # matmul output partitions (96)
PADL = 15  # left pad column; data occupies cols 16..527, right pad col 528
TW = 544  # tile slot free width (32B-aligned group stride)
BLK = 8  # groups per block (= PSUM banks used per block)
N_CORES = 8


def _weff(K: np.ndarray) -> np.ndarray:
    Kflat = K.reshape(9 * C, C).astype(np.float32)
    Weff = np.zeros((3, 3, C, C), np.float32)
    for ki in range(3):
        for kj in range(3):
            for ci in range(C):
                Weff[ki, kj, ci, :] = Kflat[ci * 9 + ki * 3 + kj, :]
    return Weff


def _build_banded_weights(K: np.ndarray):
    """lhsT matrices [3, 128, 96] fp16, hi-outer layout: k = hi*16+ci,
    m = ho*16+co, ki = hi - ho."""
    Weff = _weff(K)
    wa_hi = np.zeros((3, 128, M), np.float32)
    for kj in range(3):
        for ho in range(R):
            for ki in range(3):
                hi = ho + ki
                blk = Weff[ki, kj]  # [ci, co]
                for ci in range(C):
                    wa_hi[kj, hi * C + ci, ho * C:(ho + 1) * C] = blk[ci]
    return wa_hi.astype(np.float16)


def _group_starts(H: int):
    starts = list(range(0, H - R, R))
    if starts[-1] != H - RIN + 1:
        starts.append(H - RIN + 1)  # final overlapped group
    return starts


def build_nc(n_img: int, H: int, in_bufs: int = 8, out_bufs: int = 5,
             lookahead: int = 3, warmup: int = 14):
    HW = H * W

    nc = bacc.Bacc(None, target_bir_lowering=False)
    xs = nc.dram_tensor("xs", [n_img, H, C, W], F16, kind="ExternalInput")
    whi = nc.dram_tensor("whi", [3, 128, M], F16, kind="ExternalInput")
    ys = nc.dram_tensor("ys", [n_img, H, C, W], F16, kind="ExternalOutput")
    CW = C * W

    starts = _group_starts(H)
    # blocks of up to BLK groups; group starts within a block step by R
    # except the final overlapped group, which the AP math below does not
    # assume anything about beyond its absolute start row.
    blocks = []
    for n in range(n_img):
        i = 0
        while i < len(starts):
            chunk = starts[i:i + BLK]
            if len(chunk) > BLK // 2 and len(chunk) < BLK:
                # split the remainder so the final drain chain is short
                blocks.append((n, chunk[:BLK // 2]))
                chunk = chunk[BLK // 2:]
            blocks.append((n, chunk))
            i += BLK

    with tile.TileContext(nc) as tc:
        with (
            tc.tile_pool(name="wpool", bufs=1) as wpool,
            tc.tile_pool(name="inpool", bufs=in_bufs) as inpool,
            tc.tile_pool(name="outpool", bufs=out_bufs) as outpool,
            tc.tile_pool(name="zpool", bufs=1) as zpool,
            tc.tile_pool(name="psum", bufs=8, space="PSUM") as psum_pool,
        ):
            # weights + zero-row stores go on the (otherwise idle at start)
            # scalar queue so the sync queue issues the first loads at once
            whi_t = wpool.tile([128, 3, M], F16)
            nc.scalar.dma_start(
                whi_t[:], bass.AP(whi, 0, [[M, 128], [128 * M, 3], [1, M]])
            )

            # zero row for the masked last output row of each image;
            # these stores depend on nothing, so issue them up front
            zrow = zpool.tile([16, W], F16)
            nc.vector.memset(zrow[:], 0.0)
            for n in range(n_img):
                dst = bass.AP(ys, n * H * CW + (H - 1) * CW, [[W, C], [1, W]])
                nc.scalar.dma_start(dst, zrow[:])

            # PE p-state warmup: the tensor engine only reaches full clock
            # after ~3us of continuous execution. Run throwaway matmuls on
            # the weight tile while the first input loads are in flight so
            # the real matmul stream starts already ramped.
            if warmup:
                wa = psum_pool.tile([M, 3 * M], F32, name="warm_a", tag="ps")
                wb = psum_pool.tile([M, 3 * M], F32, name="warm_b", tag="ps")
                for i in range(warmup):
                    dst = wa if i % 2 == 0 else wb
                    nc.tensor.matmul(dst[:], whi_t[:, 0, :], whi_t[:, :, :],
                                     start=True, stop=True)

            def load_block(bi):
                """One DMA for all groups of the block: tile [128, L, TW],
                partition hi*16+ci, group g's window is rows s_g-1..s_g+6
                (one contiguous [H,C,W]-layout DRAM extent at (s_g-1)*CW).
                For s=0 the row -1 slot (partitions 0..15) is zeroed."""
                n, chunk = blocks[bi]
                L = len(chunk)
                t = inpool.tile([128, BLK, TW], F16, name=f"in_{bi}", tag="in")
                nc.gpsimd.memset(t[:, :, PADL:PADL + 1], 0.0)
                nc.gpsimd.memset(t[:, :, PADL + 1 + W:PADL + 2 + W], 0.0)
                base = n * H * CW
                if chunk[0] == 0:
                    # group 0: row -1 is zero padding
                    nc.vector.memset(t[0:16, 0, 16:16 + W], 0.0)
                    src = bass.AP(xs, base, [[W, 112], [1, W]])
                    nc.sync.dma_start(t[16:128, 0, 16:16 + W], src)
                    if L > 1:
                        src = bass.AP(
                            xs, base + (chunk[1] - 1) * CW,
                            [[W, 128], [R * CW, L - 1], [1, W]])
                        nc.sync.dma_start(t[:, 1:L, 16:16 + W], src)
                else:
                    # chunk starts step by R except possibly the last
                    # (overlapped) group; split it off if irregular.
                    L0 = L
                    if L > 1 and chunk[-1] - chunk[-2] != R:
                        L0 = L - 1
                    src = bass.AP(xs, base + (chunk[0] - 1) * CW,
                                  [[W, 128], [R * CW, L0], [1, W]])
                    nc.sync.dma_start(t[:, 0:L0, 16:16 + W], src)
                    if L0 != L:
                        src = bass.AP(xs, base + (chunk[-1] - 1) * CW,
                                      [[W, 128], [1, W]])
                        nc.sync.dma_start(t[:, L - 1, 16:16 + W], src)
                return t

            def compute_store_block(bi, t):
                n, chunk = blocks[bi]
                L = len(chunk)
                base = n * H * CW
                ps = [
                    psum_pool.tile([M, W], F32, name=f"ps_{bi}_{g}", tag="ps")
                    for g in range(L)
                ]
                for kj in range(3):
                    for g in range(L):
                        nc.tensor.matmul(
                            ps[g][:], whi_t[:, kj, :],
                            t[:, g, PADL + kj:PADL + kj + W],
                            start=(kj == 0), stop=(kj == 2),
                        )
                ot = outpool.tile([M, BLK, W], F16, name=f"out_{bi}",
                                  tag="out")
                # masked last output column: zero the col-511 stripe once
                nc.gpsimd.memset(ot[:, :, W - 1:W], 0.0)
                for g in range(L):
                    if g % 2 == 0:
                        nc.vector.tensor_copy(ot[:, g, 0:W - 1],
                                              ps[g][:, 0:W - 1])
                    else:
                        nc.scalar.copy(ot[:, g, 0:W - 1], ps[g][:, 0:W - 1])
                if L > 1 and chunk[-1] - chunk[-2] != R:
                    # regular prefix in one DMA, overlapped tail separately
                    dst = bass.AP(ys, base + chunk[0] * CW,
                                  [[W, M], [R * CW, L - 1], [1, W]])
                    nc.scalar.dma_start(dst, ot[:, 0:L - 1, :])
                    dst = bass.AP(ys, base + chunk[-1] * CW, [[W, M], [1, W]])
                    nc.scalar.dma_start(dst, ot[:, L - 1, :])
                else:
                    dst = bass.AP(ys, base + chunk[0] * CW,
                                  [[W, M], [R * CW, L], [1, W]])
                    nc.scalar.dma_start(dst, ot[:, 0:L, :])

            pending = []
            for bi in range(len(blocks) + lookahead):
                if bi < len(blocks):
                    pending.append(load_block(bi))
                if bi >= lookahead:
                    compute_store_block(bi - lookahead, pending.pop(0))

    nc.finalize()
    return nc


def _run(x: np.ndarray, K: np.ndarray, core_ids, trace=False, **kw):
    """x: [n_total, C, H, W] fp32, split evenly over core_ids."""
    n_cores = len(core_ids)
    n_total = x.shape[0]
    assert n_total % n_cores == 0
    n_per = n_total // n_cores
    H = x.shape[2]
    wa_hi = _build_banded_weights(K)
    # device layout is [n, H, C, W]
    x16 = np.ascontiguousarray(x.astype(np.float16).transpose(0, 2, 1, 3))
    nc = build_nc(n_per, H, **kw)
    in_maps = [
        {
            "xs": np.ascontiguousarray(x16[i * n_per:(i + 1) * n_per]),
            "whi": wa_hi,
        }
        for i in range(n_cores)
    ]
    res = run_bass_kernel_spmd(nc, in_maps, core_ids=list(core_ids),
                               trace=trace)
    y = np.concatenate([r["ys"] for r in res.results], axis=0)
    y = np.ascontiguousarray(y.transpose(0, 2, 1, 3).astype(np.float32))
    return y, res


def kernel(**inputs) -> np.ndarray:
    x = np.ascontiguousarray(np.asarray(inputs["x"], dtype=np.float32))
    K = np.ascontiguousarray(np.asarray(inputs["K"], dtype=np.float32))
    y, _ = _run(x, K, core_ids=range(N_CORES))
    return y


# revision 18
# speedup vs baseline: 1.0329x; 1.0329x over previous
"""Trainium2 Bass kernel for a 3x3 stride-1 pad-1 conv, NCHW (16,16,512,512) fp32.

Matches the reference semantics exactly:
  - effective weights: K flattened as (ki,kj,ci) but consumed as (ci,ki,kj):
      Weff[ki,kj,ci,co] = K.reshape(144,16)[ci*9 + ki*3 + kj, co]
  - last output row and column are zero.

Strategy: pure data parallel over the batch (2 images per core on 8 cores),
weights replicated. x is staged to the device as fp16 (host cast) and the
output is produced as fp16 on device (host upcast), halving both directions
of HBM traffic; accumulation stays fp32 in PSUM. Both device-side tensors
use a host-transposed [n, H, C, W] layout: the DRAM address of SBUF
partition p = hi*16+ci (resp. ho*16+co) is then p*W, a single
uniform-stride AP dim, which lets one 3-dim DMA move a whole block of
groups and makes every group window a contiguous 128KB DRAM extent.

Per core the conv runs as banded fp16 matmuls on the tensor engine:
  - output rows in groups of R=6; contraction K = 8 input rows x 16 c_in = 128
    partitions (rows s-1 .. s+6), M = 6 out rows x 16 c_out = 96;
  - partition layout hi*16+ci; data cols 16..527 of a 544-wide padded slot,
    pad cols 15/528 memset to zero;
  - the 3 kj taps are column-shifted slices of the padded row tile;
  - group starts: 0, 6, ..., 504, then a final overlapped group at 505
    (recomputed rows store near-identical bytes, so the overlap is benign);
  - groups are processed in blocks of 8: ONE input DMA loads all 8 group
    windows (the HWDGE issue path is a serialized ~630ns/instruction shared
    resource, so few fat DMAs beat many thin ones), 24 matmuls run
    back-to-back into 8 PSUM banks (keeping the PE continuously busy so it
    stays at its full-speed p-state), PSUM is evicted with an fp32->fp16
    cast split across the vector and scalar engines into an fp16 staging
    tile, and ONE store DMA scatters the whole block to DRAM.
"""

import numpy as np

import concourse.bass as bass
import concourse.mybir as mybir
import concourse.tile as tile
from concourse import bacc
from concourse.bass_utils import run_bass_kernel_spmd

F32 = mybir.dt.float32
F16 = mybir.dt.float16

C = 16  # channels (in == out)
W = 512  # image width
R = 6  # output rows per matmul group
RIN = R + 2  # input rows per group
M = R * C  # matmul output partitions (96)
PADL = 15  # left pad column; data occupies cols 16..527, right pad col 528
TW = 544  # tile slot free width (32B-aligned group stride)
BLK = 8  # groups per block (= PSUM banks used per block)
N_CORES = 8


def _weff(K: np.ndarray) -> np.ndarray:
    Kflat = K.reshape(9 * C, C).astype(np.float32)
    Weff = np.zeros((3, 3, C, C), np.float32)
    for ki in range(3):
        for kj in range(3):
            for ci in range(C):
                Weff[ki, kj, ci, :] = Kflat[ci * 9 + ki * 3 + kj, :]
    return Weff


def _build_banded_weights(K: np.ndarray):
    """lhsT matrices [3, 128, 96] fp16, hi-outer layout: k = hi*16+ci,
    m = ho*16+co, ki = hi - ho."""
    Weff = _weff(K)
    wa_hi = np.zeros((3, 128, M), np.float32)
    for kj in range(3):
        for ho in range(R):
            for ki in range(3):
                hi = ho + ki
                blk = Weff[ki, kj]  # [ci, co]
                for ci in range(C):
                    wa_hi[kj, hi * C + ci, ho * C:(ho + 1) * C] = blk[ci]
    return wa_hi.astype(np.float16)


def _group_starts(H: int):
    starts = list(range(0, H - R, R))
    if starts[-1] != H - RIN + 1:
        starts.append(H - RIN + 1)  # final overlapped group
    return starts


def build_nc(n_img: int, H: int, in_bufs: int = 8, out_bufs: int = 5,
             lookahead: int = 4):
    HW = H * W

    nc = bacc.Bacc(None, target_bir_lowering=False)
    xs = nc.dram_tensor("xs", [n_img, H, C, W], F16, kind="ExternalInput")
    whi = nc.dram_tensor("whi", [3, 128, M], F16, kind="ExternalInput")
    ys = nc.dram_tensor("ys", [n_img, H, C, W], F16, kind="ExternalOutput")
    CW = C * W

    starts = _group_starts(H)
    # Blocks of up to BLK groups; group starts within a block step by R
    # except the final overlapped group, which the AP math below does not
    # assume anything about beyond its absolute start row.
    # The first image ramps up with small blocks (fast pipeline fill) and
    # the last image ramps down (short drain chain); sizes sum to
    # len(starts) = 86 per image.
    n_groups = len(starts)
    ramp = [2, 4]
    assert (n_groups - sum(ramp)) % BLK == 0
    full = [BLK] * ((n_groups - sum(ramp)) // BLK)
    blocks = []
    for n in range(n_img):
        if n == 0:
            sizes = ramp + full
        elif n == n_img - 1:
            sizes = full + ramp[::-1]
        else:
            sizes = [BLK] * (n_groups // BLK) + [n_groups % BLK]
        i = 0
        for sz in sizes:
            blocks.append((n, starts[i:i + sz]))
            i += sz
        assert i == n_groups

    with tile.TileContext(nc) as tc:
        with (
            tc.tile_pool(name="wpool", bufs=1) as wpool,
            tc.tile_pool(name="inpool", bufs=in_bufs) as inpool,
            tc.tile_pool(name="outpool", bufs=out_bufs) as outpool,
            tc.tile_pool(name="zpool", bufs=1) as zpool,
            tc.tile_pool(name="psum", bufs=8, space="PSUM") as psum_pool,
        ):
            # weights + zero-row stores go on the (otherwise idle at start)
            # scalar queue so the sync queue issues the first loads at once
            whi_t = wpool.tile([128, 3, M], F16)
            nc.scalar.dma_start(
                whi_t[:], bass.AP(whi, 0, [[M, 128], [128 * M, 3], [1, M]])
            )

            # zero row for the masked last output row of each image;
            # these stores depend on nothing, so issue them up front
            zrow = zpool.tile([16, W], F16)
            nc.vector.memset(zrow[:], 0.0)
            for n in range(n_img):
                dst = bass.AP(ys, n * H * CW + (H - 1) * CW, [[W, C], [1, W]])
                nc.scalar.dma_start(dst, zrow[:])



            def load_block(bi):
                """One DMA for all groups of the block: tile [128, L, TW],
                partition hi*16+ci, group g's window is rows s_g-1..s_g+6
                (one contiguous [H,C,W]-layout DRAM extent at (s_g-1)*CW).
                For s=0 the row -1 slot (partitions 0..15) is zeroed."""
                n, chunk = blocks[bi]
                L = len(chunk)
                t = inpool.tile([128, BLK, TW], F16, name=f"in_{bi}", tag="in")
                nc.gpsimd.memset(t[:, :, PADL:PADL + 1], 0.0)
                nc.gpsimd.memset(t[:, :, PADL + 1 + W:PADL + 2 + W], 0.0)
                base = n * H * CW
                if chunk[0] == 0:
                    # group 0: row -1 is zero padding
                    nc.vector.memset(t[0:16, 0, 16:16 + W], 0.0)
                    src = bass.AP(xs, base, [[W, 112], [1, W]])
                    nc.sync.dma_start(t[16:128, 0, 16:16 + W], src)
                    if L > 1:
                        src = bass.AP(
                            xs, base + (chunk[1] - 1) * CW,
                            [[W, 128], [R * CW, L - 1], [1, W]])
                        nc.sync.dma_start(t[:, 1:L, 16:16 + W], src)
                else:
                    # chunk starts step by R except possibly the last
                    # (overlapped) group; split it off if irregular.
                    L0 = L
                    if L > 1 and chunk[-1] - chunk[-2] != R:
                        L0 = L - 1
                    src = bass.AP(xs, base + (chunk[0] - 1) * CW,
                                  [[W, 128], [R * CW, L0], [1, W]])
                    nc.sync.dma_start(t[:, 0:L0, 16:16 + W], src)
                    if L0 != L:
                        src = bass.AP(xs, base + (chunk[-1] - 1) * CW,
                                      [[W, 128], [1, W]])
                        nc.sync.dma_start(t[:, L - 1, 16:16 + W], src)
                return t

            def compute_store_block(bi, t):
                n, chunk = blocks[bi]
                L = len(chunk)
                base = n * H * CW
                ps = [
                    psum_pool.tile([M, W], F32, name=f"ps_{bi}_{g}", tag="ps")
                    for g in range(L)
                ]
                for kj in range(3):
                    for g in range(L):
                        nc.tensor.matmul(
                            ps[g][:], whi_t[:, kj, :],
                            t[:, g, PADL + kj:PADL + kj + W],
                            start=(kj == 0), stop=(kj == 2),
                        )
                ot = outpool.tile([M, BLK, W], F16, name=f"out_{bi}",
                                  tag="out")
                # masked last output column: zero the col-511 stripe once
                nc.gpsimd.memset(ot[:, :, W - 1:W], 0.0)
                for g in range(L):
                    if g % 2 == 0:
                        nc.vector.tensor_copy(ot[:, g, 0:W - 1],
                                              ps[g][:, 0:W - 1])
                    else:
                        nc.scalar.copy(ot[:, g, 0:W - 1], ps[g][:, 0:W - 1])
                if L > 1 and chunk[-1] - chunk[-2] != R:
                    # regular prefix in one DMA, overlapped tail separately
                    dst = bass.AP(ys, base + chunk[0] * CW,
                                  [[W, M], [R * CW, L - 1], [1, W]])
                    nc.scalar.dma_start(dst, ot[:, 0:L - 1, :])
                    dst = bass.AP(ys, base + chunk[-1] * CW, [[W, M], [1, W]])
                    nc.scalar.dma_start(dst, ot[:, L - 1, :])
                else:
                    dst = bass.AP(ys, base + chunk[0] * CW,
                                  [[W, M], [R * CW, L], [1, W]])
                    nc.scalar.dma_start(dst, ot[:, 0:L, :])

            pending = []
            for bi in range(len(blocks) + lookahead):
                if bi < len(blocks):
                    pending.append(load_block(bi))
                if bi >= lookahead:
                    compute_store_block(bi - lookahead, pending.pop(0))

    nc.finalize()
    return nc


def _run(x: np.ndarray, K: np.ndarray, core_ids, trace=False, **kw):
    """x: [n_total, C, H, W] fp32, split evenly over core_ids."""
    n_cores = len(core_ids)
    n_total = x.shape[0]
    assert n_total % n_cores == 0
    n_per = n_total // n_cores
    H = x.shape[2]
    wa_hi = _build_banded_weights(K)
    # device layout is [n, H, C, W]
    x16 = np.ascontiguousarray(x.astype(np.float16).transpose(0, 2, 1, 3))
    nc = build_nc(n_per, H, **kw)
    in_maps = [
        {
            "xs": np.ascontiguousarray(x16[i * n_per:(i + 1) * n_per]),
            "whi": wa_hi,
        }
        for i in range(n_cores)
    ]
    res = run_bass_kernel_spmd(nc, in_maps, core_ids=list(core_ids),
                               trace=trace)
    y = np.concatenate([r["ys"] for r in res.results], axis=0)
    y = np.ascontiguousarray(y.transpose(0, 2, 1, 3).astype(np.float32))
    return y, res


def kernel(**inputs) -> np.ndarray:
    x = np.ascontiguousarray(np.asarray(inputs["x"], dtype=np.float32))
    K = np.ascontiguousarray(np.asarray(inputs["K"], dtype=np.float32))
    y, _ = _run(x, K, core_ids=range(N_CORES))
    return y
